# revision 26
# baseline (speedup 1.0000x reference)
"""Multi-head self-attention with RoPE (B=2, S=2048, D=1024, H=16, d_k=64,
causal) on 8 trn2 NeuronCores.

Sharding: core c -> batch c//4, heads [4*(c%4), 4*(c%4)+4). Each core gets
x[b]^T, its 4 heads' slices of Wq/Wk/Wv (output dim) and Wo (input dim),
computes a partial y^T = Wo_slice^T . attn_out^T, and the host sums the 4
partials per batch.

Device kernel (per core; matmul operands bf16 by default, f32 PSUM accum):
  1. QKV projection from x^T (model dim on partitions) producing Q^T/K^T
     (head-d on partitions, 2 heads stacked per 128) and V (seq on
     partitions). RoPE applied to Q^T/K^T as q*cos + R^T(q*sin) where R is a
     signed permutation matmul; the head-d axis is pre-permuted (host side)
     to block-of-32 layout so cos/sin rows are partition-aligned.
  2. Transposed-flash attention per (head, 1024-wide q window), k-outer,
     software-pipelined (scores/exp of tile kt+1 overlap attnV of kt):
     scores^T[k,q] = K_tile^T.T @ Q^T (k on partitions), one exp on ACT
     (scale=1/8) over the valid q range, triangular mask multiply on
     diagonal tiles, then attnV out^T[d,q] += V'[k,:].T @ P^T accumulated in
     two half-window [128,512] PSUM tiles. V' carries a ones column (and,
     for odd heads, 64 leading pad columns) so the softmax denominator
     accumulates in a spare PSUM row and odd heads land on partitions
     64..127 directly. Each half normalizes as soon as its k range
     completes: DVE reciprocal of the denominator row -> SBUF, DMA
     partition-broadcast, one DVE multiply into out^T.
  3. y^T[o,s] = Wo^T.T @ out^T, DMA out.
  Phase-1 work for s-chunks 2,3 is interleaved into attention window 0 and
  phase 3 for window w-1 into window w, so PE fills ACT-bound stretches.
"""
import os
import sys

import numpy as np

sys.path.insert(0, "/opt/trn_rl_repo")

D_MODEL = 1024
NUM_HEADS = 16
DK = 64
B = 2
S = 2048
THETA = 10000.0
NCORES = 8
HPC = 4          # heads per core
NPAIRS = 2       # head pairs per core
KT = 128         # k tile (partition dim of scores^T)
QW = 1024        # q window
NW = S // QW     # q windows
NI = D_MODEL // 128   # i (contraction) tiles for projections
NCHUNK = S // 512     # 512-wide s chunks

# V tile layout per head group: [65 | 128 | 65 | 128] columns.
# Even local heads: 64 d columns then a ones column (denominator lands in
# PSUM row 64). Odd local heads: 32 zero cols, ones col, 31 zero cols, then
# 64 d columns -- so attnV output rows are 64..127 (matching oT's lower
# half) and the denominator lands in (32-aligned) PSUM row 32.
VW = 386
V_SLICE = ((0, 65), (65, 193), (193, 258), (258, 386))
V_DEN_ROW = (64, 32)  # PSUM row holding the denominator, per half

_prog = {}


def _mm_mode():
    return os.environ.get("MHA_MM_DTYPE", "bf16")


def _install_hook_wrapper(bass2jax):
    """Install the neuronx compile hook with a traceback printer (the PJRT
    layer swallows python exceptions from the hook)."""
    import traceback

    bass2jax.install_neuronx_cc_hook()
    import libneuronxla

    if getattr(libneuronxla, "_mha_wrapped", False):
        return
    orig = libneuronxla.neuronx_cc

    def wrapped(*a, **k):
        try:
            return orig(*a, **k)
        except Exception:
            traceback.print_exc()
            raise

    libneuronxla.neuronx_cc = wrapped
    libneuronxla._mha_wrapped = True
    bass2jax.install_neuronx_cc_hook = lambda: None


def _split_excess_waits(nc, max_waits=1):
    """This container's walrus accepts at most one sync-wait per
    instruction; redistribute extras onto same-engine NOPs inserted just
    before the offending instruction."""
    import bass_rust
    import concourse.mybir as mybir

    counter = [0]
    for fn in nc.m.functions:
        for bb in fn.blocks:
            out = []
            changed = False
            for inst in bb.instructions:
                si = inst.sync_info
                waits = list(si.on_wait) if si is not None and si.on_wait else []
                if len(waits) > max_waits:
                    changed = True
                    keep = waits[-max_waits:]
                    extras = waits[:-max_waits]
                    for i in range(0, len(extras), max_waits):
                        counter[0] += 1
                        nop = mybir.InstNoOp(
                            name=f"I-waitsplit-{counter[0]}",
                            ins=[],
                            outs=[],
                            engine=inst.engine,
                        )
                        nop.sync_info = bass_rust.SyncInfo(
                            on_wait=extras[i : i + max_waits], on_update=[]
                        )
                        out.append(nop)
                    si.on_wait = keep
                out.append(inst)
            if changed:
                bb.instructions = out


def _build_program(split_waits=True):
    import concourse.bass as bass
    import concourse.mybir as mybir
    from concourse import tile

    F32 = mybir.dt.float32
    mode = _mm_mode()
    MM = {
        "bf16": mybir.dt.bfloat16,
        "f32r": mybir.dt.float32r,
        "f32": mybir.dt.float32,
    }[mode]
    AF = mybir.ActivationFunctionType
    ALU = mybir.AluOpType

    nc = bass.Bass(target_bir_lowering=False, trn_type="TRN2")

    F16 = mybir.dt.float16
    xt = nc.dram_tensor("xt", [D_MODEL, S], MM, kind="ExternalInput")
    # wq/wk/wv packed i-major: [:, 256*i : 256*(i+1)] is contraction tile i,
    # making each weight load a single fully-contiguous DMA
    wqt = nc.dram_tensor("wqt", [128, 2048], MM, kind="ExternalInput")
    wkt = nc.dram_tensor("wkt", [128, 2048], MM, kind="ExternalInput")
    wvt = nc.dram_tensor("wvt", [128, 2048], MM, kind="ExternalInput")
    # cos | sin | rsign | masku packed into one [128, 4352] tensor; Wo^T
    # packed to [128, 2048] (pair-major) -- single contiguous DMAs
    wot = nc.dram_tensor("wot", [128, 2 * D_MODEL], MM, kind="ExternalInput")
    csb = nc.dram_tensor("csb", [128, 2 * S + 256], MM, kind="ExternalInput")
    yt = nc.dram_tensor("yt", [D_MODEL, S], F16, kind="ExternalOutput")

    with tile.TileContext(nc) as tc:
        with (
            tc.tile_pool(name="const", bufs=1) as cp,
            tc.tile_pool(name="xtp", bufs=24) as xtp,
            tc.tile_pool(name="work", bufs=3) as wk,
            tc.tile_pool(name="nrm", bufs=4) as nrm,
            tc.tile_pool(name="bcp", bufs=4) as bcp,
            tc.tile_pool(name="pT", bufs=4) as pTp,
            tc.tile_pool(name="yp", bufs=5) as yp,
            tc.tile_pool(name="psS", bufs=2, space="PSUM") as psS,
            tc.tile_pool(name="psW", bufs=4, space="PSUM") as psW,
        ):
            # ---- DMAs: wq & x chunk 0 first (first-matmul critical path),
            # then rope consts, wk, wv, mask, wo; x chunks 1-3 prefetch ----
            w_pack = {}
            def in_dma(n, out, in_):
                nc.sync.dma_start(out=out, in_=in_)

            wq_sb = cp.tile([128, 2048], MM, tag="wq")
            in_dma(0, wq_sb[:, 0:1024], wqt[:, 0:1024])
            in_dma(0, wq_sb[:, 1024:2048], wqt[:, 1024:2048])
            w_pack["q"] = wq_sb
            wk_early = True
            # x loaded as [128,1024] halves (2KB DMA lines); chunk views below
            x_half = [[None] * NI for _ in range(2)]
            cs_t = cp.tile([128, 2 * S + 256], MM, tag="csb")
            cos_sb = cs_t[:, 0:S]
            sin_sb = cs_t[:, S : 2 * S]
            r_sb = cs_t[:, 2 * S : 2 * S + 128]
            m_sb = cs_t[:, 2 * S + 128 : 2 * S + 256]
            wk_sb = cp.tile([128, 2048], MM, tag="wk")
            n_dma = 1
            in_dma(n_dma, wk_sb[:], wkt[:])
            n_dma += 1
            for i in range(NI):
                t = xtp.tile([128, 1024], MM, tag="xt", name="xh0")
                in_dma(n_dma, t[:], xt[128 * i : 128 * i + 128, 0:1024])
                x_half[0][i] = t
                n_dma += 1
            in_dma(n_dma, cs_t[:], csb[:])
            n_dma += 1
            w_pack["k"] = wk_sb
            wv_sb = cp.tile([128, 2048], MM, tag="wv")
            in_dma(n_dma, wv_sb[:], wvt[:])
            n_dma += 1
            w_pack["v"] = wv_sb
            wo_t = cp.tile([128, 2 * D_MODEL], MM, tag="wo")
            in_dma(n_dma, wo_t[:], wot[:])
            n_dma += 1
            wo_sb = [wo_t[:, D_MODEL * p : D_MODEL * p + D_MODEL] for p in range(NPAIRS)]
            for i in range(NI):
                t = xtp.tile([128, 1024], MM, tag="xt", name="xh1")
                in_dma(n_dma, t[:], xt[128 * i : 128 * i + 128, 1024:2048])
                n_dma += 1
                x_half[1][i] = t

            def w_tile(name, i):
                return w_pack[name][:, 256 * i : 256 * i + 256]

            def x_tile(c, i):
                return x_half[c // 2][i][:, 512 * (c % 2) : 512 * (c % 2) + 512]

            qT_sb = [cp.tile([128, S], MM, tag=f"qT{p}", name=f"qT{p}") for p in range(NPAIRS)]
            kT_sb = [cp.tile([128, S], MM, tag=f"kT{p}", name=f"kT{p}") for p in range(NPAIRS)]
            oT_sb = [cp.tile([128, S], MM, tag=f"oT{p}", name=f"oT{p}") for p in range(NPAIRS)]
            v_sb = [cp.tile([128, VW], MM, tag=f"v{j}", name=f"v{j}") for j in range(S // KT)]

            # ---- phase 1 pieces ----
            def qk_chunk(c, p, name, dst):
                sc = slice(512 * c, 512 * c + 512)
                pc = slice(128 * p, 128 * p + 128)
                ps = psW.tile([128, 512], F32, tag="w")
                for i in range(NI):
                    nc.tensor.matmul(
                        out=ps[:],
                        lhsT=w_tile(name, i)[:, pc],
                        rhs=x_tile(c, i),
                        start=(i == 0),
                        stop=(i == NI - 1),
                    )
                tsin = wk.tile([128, 512], MM, tag="tsin")
                nc.vector.tensor_tensor(
                    out=tsin[:], in0=ps[:], in1=sin_sb[:, sc], op=ALU.mult
                )
                tcos = wk.tile([128, 512], F32, tag="tcos")
                nc.vector.tensor_tensor(
                    out=tcos[:], in0=ps[:], in1=cos_sb[:, sc], op=ALU.mult
                )
                pssh = psW.tile([128, 512], F32, tag="w")
                nc.tensor.matmul(
                    out=pssh[:], lhsT=r_sb, rhs=tsin[:], start=True, stop=True
                )
                nc.vector.tensor_tensor(
                    out=dst[p][:, sc], in0=pssh[:], in1=tcos[:], op=ALU.add
                )

            def v_chunk_tile(c, st):
                j = 4 * c + st
                stl = slice(128 * st, 128 * st + 128)
                psv = psW.tile([128, 256], F32, tag="w")
                for i in range(NI):
                    nc.tensor.matmul(
                        out=psv[:],
                        lhsT=x_tile(c, i)[:, stl],
                        rhs=w_tile("v", i),
                        start=(i == 0),
                        stop=(i == NI - 1),
                    )
                vt = v_sb[j]
                base = vt[:]
                # odd-head prefix: zeros (64 cols), then ones at local col 32
                nc.vector.memset(
                    bass.AP(base.tensor, base.offset + 65, [[VW, 128], [193, 2], [1, 64]]),
                    0.0,
                )
                nc.vector.memset(
                    bass.AP(base.tensor, base.offset + 97, [[VW, 128], [193, 2]]), 1.0
                )
                # even-head ones column (col 64 of the 65-wide slices)
                nc.vector.memset(
                    bass.AP(base.tensor, base.offset + 64, [[VW, 128], [193, 2]]), 1.0
                )
                pv = psv[:]
                # d columns: even halves (offsets 0, 193), odd halves (129, 322)
                nc.vector.tensor_copy(
                    out=bass.AP(base.tensor, base.offset + 0, [[VW, 128], [193, 2], [1, 64]]),
                    in_=bass.AP(pv.tensor, pv.offset + 0, [[256, 128], [128, 2], [1, 64]]),
                )
                nc.vector.tensor_copy(
                    out=bass.AP(base.tensor, base.offset + 129, [[VW, 128], [193, 2], [1, 64]]),
                    in_=bass.AP(pv.tensor, pv.offset + 64, [[256, 128], [128, 2], [1, 64]]),
                )

            # ---- attention ----
            def normalize(w, h, acc_t, beta):
                p, half = divmod(h, 2)
                qs = slice(QW * w + 512 * beta, QW * w + 512 * beta + 512)
                dr = 64 * half  # d-row base in acc/oT
                den = nrm.tile([1, 512], F32, tag="den")
                drow = V_DEN_ROW[half]
                nc.vector.reciprocal(
                    out=den[:], in_=acc_t[drow : drow + 1, :]
                )
                bc = bcp.tile([128, 512], F32, tag="bc")
                dap = den[:]
                nc.gpsimd.dma_start(
                    out=bc[dr : dr + 64, :],
                    in_=bass.AP(dap.tensor, dap.offset, [[512, 1], [0, 64], [1, 512]]),
                )
                nc.vector.tensor_tensor(
                    out=oT_sb[p][dr : dr + 64, qs],
                    in0=acc_t[dr : dr + 64, :],
                    in1=bc[dr : dr + 64, :],
                    op=ALU.mult,
                )

            def attn_head(w, h, post_left=None, post_right=None):
                p, half = divmod(h, 2)
                pr = slice(64 * half, 64 * half + 64)
                a0, a1 = V_SLICE[h]
                q0 = QW * w
                acc = [
                    psW.tile([128, 512], F32, tag="w", name="accL"),
                    psW.tile([128, 512], F32, tag="w", name="accR"),
                ]
                kmax = (QW // KT) * (w + 1)
                left_stop = (QW // KT) * w + 3
                pend = None  # software pipeline: attnV trails scores/exp by one

                def attn_v(kt, pT):
                    k0 = KT * kt
                    qoff = max(k0 - q0, 0)
                    subs = [(qoff, 512), (512, QW)] if qoff < 512 else [(qoff, QW)]
                    for a, b in subs:
                        beta = a // 512
                        nc.tensor.matmul(
                            out=acc[beta][0 : a1 - a0, a - 512 * beta : b - 512 * beta],
                            lhsT=v_sb[kt][:, a0:a1],
                            rhs=pT[:, a:b],
                            start=(kt == 0),
                            stop=(kt == left_stop + 4 * beta),
                        )
                    if kt == left_stop:
                        normalize(w, h, acc[0], 0)
                        if post_left:
                            post_left()

                for kt in range(kmax):
                    k0 = KT * kt
                    qoff = max(k0 - q0, 0)
                    ps_s = psS.tile([128, QW], F32, tag="s")
                    subs = [(qoff, 512), (512, QW)] if qoff < 512 else [(qoff, QW)]
                    for a, b in subs:
                        nc.tensor.matmul(
                            out=ps_s[:, a:b],
                            lhsT=kT_sb[p][pr, k0 : k0 + KT],
                            rhs=qT_sb[p][pr, q0 + a : q0 + b],
                            start=True,
                            stop=True,
                        )
                    pT = pTp.tile([128, QW], MM, tag="pT")
                    nc.scalar.activation(
                        out=pT[:, qoff:QW], in_=ps_s[:, qoff:QW], func=AF.Exp, scale=0.125
                    )
                    if k0 >= q0:
                        nc.vector.tensor_tensor(
                            out=pT[:, qoff : qoff + KT],
                            in0=pT[:, qoff : qoff + KT],
                            in1=m_sb,
                            op=ALU.mult,
                        )
                    if pend is not None:
                        attn_v(*pend)
                    pend = (kt, pT)
                attn_v(*pend)
                normalize(w, h, acc[1], 1)
                if post_right:
                    post_right()

            # ---- phase 3 ----
            def emit_phase3_chunk(c, ocr, use_psS, act_alt):
                # phase 3 for 512-col chunk c, output rows in `ocr`.
                # use_psS: also draw PSUM slots from the (drained) scores
                # pool. act_alt: alternate copies ACT/DVE (else DVE only --
                # used while ACT is still exp-bound).
                sc = slice(512 * c, 512 * c + 512)
                for n_item, oc in enumerate(ocr):
                    ocs = slice(128 * oc, 128 * oc + 128)
                    if use_psS and n_item % 3 == 2:
                        ps_y = psS.tile([128, 512], F32, tag="s", name="ps_ys")
                    else:
                        ps_y = psW.tile([128, 512], F32, tag="w", name="ps_y")
                    for p in range(NPAIRS):
                        nc.tensor.matmul(
                            out=ps_y[:],
                            lhsT=wo_sb[p][:, ocs],
                            rhs=oT_sb[p][:, sc],
                            start=(p == 0),
                            stop=(p == NPAIRS - 1),
                        )
                    y_sb = yp.tile([128, 512], F16, tag="y", name="y_sb")
                    if act_alt and n_item % 2 == 0:
                        nc.scalar.activation(out=y_sb[:], in_=ps_y[:], func=AF.Copy)
                    else:
                        nc.vector.tensor_copy(out=y_sb[:], in_=ps_y[:])
                    nc.sync.dma_start(out=yt[ocs, sc], in_=y_sb[:])

            def emit_phase3(win, part=None, chunks=None, items=None, act_copy=False):
                if items is None:
                    items = [
                        (c, oc)
                        for c in (chunks if chunks is not None else (2 * win, 2 * win + 1))
                        for oc in range(D_MODEL // 128)
                    ]
                    if part is not None:
                        items = items[4 * part : 4 * part + 4]
                for n_item, (c, oc) in enumerate(items):
                    sc = slice(512 * c, 512 * c + 512)
                    ocs = slice(128 * oc, 128 * oc + 128)
                    ps_y = psW.tile([128, 512], F32, tag="w", name="ps_y")
                    for p in range(NPAIRS):
                        nc.tensor.matmul(
                            out=ps_y[:],
                            lhsT=wo_sb[p][:, ocs],
                            rhs=oT_sb[p][:, sc],
                            start=(p == 0),
                            stop=(p == NPAIRS - 1),
                        )
                    y_sb = yp.tile([128, 512], F16, tag="y", name="y_sb")
                    if act_copy and n_item % 2 == 0:
                        nc.scalar.activation(
                            out=y_sb[:], in_=ps_y[:], func=AF.Copy
                        )
                    else:
                        nc.vector.tensor_copy(out=y_sb[:], in_=ps_y[:])
                    nc.sync.dma_start(out=yt[ocs, sc], in_=y_sb[:])

            # ---- schedule ----
            for c in (0, 1):
                for p in range(NPAIRS):
                    qk_chunk(c, p, "q", qT_sb)
                    qk_chunk(c, p, "k", kT_sb)
                for st in range(4):
                    v_chunk_tile(c, st)

            def fill(idx):
                # phase-1 work for chunks 2,3, slotted into window-0 attention
                if idx == 0:
                    qk_chunk(2, 0, "q", qT_sb)
                    qk_chunk(2, 0, "k", kT_sb)
                elif idx == 1:
                    qk_chunk(2, 1, "q", qT_sb)
                    qk_chunk(2, 1, "k", kT_sb)
                    for st in range(4):
                        v_chunk_tile(2, st)
                elif idx == 2:
                    qk_chunk(3, 0, "q", qT_sb)
                    qk_chunk(3, 0, "k", kT_sb)
                    for st in range(4):
                        v_chunk_tile(3, st)
                else:
                    qk_chunk(3, 1, "q", qT_sb)
                    qk_chunk(3, 1, "k", kT_sb)

            horder = (1, 3, 0, 2)
            for idx, h in enumerate(horder):
                attn_head(0, h)
                fill(idx)
            p30 = [(c, oc) for c in (0, 1) for oc in range(D_MODEL // 128)]
            for idx, h in enumerate(horder):
                last = idx == len(horder) - 1
                mine = p30[4 * idx : 4 * idx + 4]

                def post_left(mine=mine):
                    emit_phase3(0, items=mine[:2])

                def post_right(mine=mine, last=last):
                    emit_phase3(0, items=mine[2:])
                    if last:
                        emit_phase3_chunk(2, range(8), use_psS=True, act_alt=True)

                attn_head(1, h, post_left=post_left, post_right=post_right)
            emit_phase3_chunk(3, range(8), use_psS=True, act_alt=True)

    if split_waits:
        _split_excess_waits(nc)
    return nc


def _get_program():
    if "nc" not in _prog:
        from concourse import bass2jax

        _install_hook_wrapper(bass2jax)
        _prog["nc"] = _build_program()
    return _prog["nc"]


def _perm_rows(g):
    """DRAM row order of Wq/Wk for core head-group g: pair-major, head-major,
    evens-then-odds within each head's 64 dims."""
    perm64 = list(range(0, 64, 2)) + list(range(1, 64, 2))
    rows = []
    for h in range(HPC):
        head = HPC * g + h
        rows += [64 * head + j for j in perm64]
    return rows


def _plain_rows(g):
    return [64 * (HPC * g) + j for j in range(64 * HPC)]


def _np_mm():
    if _mm_mode() == "bf16":
        import ml_dtypes

        return ml_dtypes.bfloat16
    return np.float32


def _host_inputs(x, token_positions, Wq, Wk, Wv, Wo):
    mmt = _np_mm()
    x = np.asarray(x, dtype=np.float32)
    pos = np.asarray(token_positions).astype(np.float64)
    Wq = np.asarray(Wq, dtype=np.float32)
    Wk = np.asarray(Wk, dtype=np.float32)
    Wv = np.asarray(Wv, dtype=np.float32)
    Wo = np.asarray(Wo, dtype=np.float32)

    inv = 1.0 / THETA ** (np.arange(0, DK, 2, dtype=np.float64) / DK)
    ang = pos[:, None] * inv[None, :]          # (S, 32)
    cosb = np.tile(np.cos(ang).T.astype(np.float32), (4, 1))  # (128, S)
    sinb = np.tile(np.sin(ang).T.astype(np.float32), (4, 1))

    rsign = np.zeros((128, 128), dtype=np.float32)
    j = np.arange(32)
    for blk in range(2):
        o = 64 * blk
        rsign[o + 32 + j, o + j] = -1.0
        rsign[o + j, o + 32 + j] = 1.0
    masku = np.triu(np.ones((128, 128), dtype=np.float32))

    def _pack(wt):  # (1024, 256) -> (128, 2048), i-major contraction tiles
        return np.ascontiguousarray(
            wt.reshape(8, 128, 256).transpose(1, 0, 2).reshape(128, 2048)
        )

    in_maps = []
    for c in range(NCORES):
        b, g = divmod(c, 4)
        rows = _perm_rows(g)
        vrows = _plain_rows(g)
        in_maps.append(
            {
                "xt": np.ascontiguousarray(x[b].T).astype(mmt),
                "wqt": _pack(Wq[rows, :].T).astype(mmt),
                "wkt": _pack(Wk[rows, :].T).astype(mmt),
                "wvt": _pack(Wv[vrows, :].T).astype(mmt),
                "wot": np.ascontiguousarray(
                    np.concatenate(
                        [Wo[:, vrows].T[128 * p : 128 * p + 128, :] for p in range(2)],
                        axis=1,
                    )
                ).astype(mmt),
                "csb": np.ascontiguousarray(
                    np.concatenate([cosb, sinb, rsign, masku], axis=1)
                ).astype(mmt),
            }
        )
    return in_maps


def run_sharded(x, token_positions, Wq, Wk, Wv, Wo, trace=False):
    from concourse.bass_utils import run_bass_kernel_spmd

    nc = _get_program()
    in_maps = _host_inputs(x, token_positions, Wq, Wk, Wv, Wo)
    res = run_bass_kernel_spmd(
        nc, in_maps, list(range(NCORES)), trace=trace
    )
    y = np.zeros((B, S, D_MODEL), dtype=np.float32)
    for c in range(NCORES):
        y[c // 4] += res.results[c]["yt"].T.astype(np.float32)
    return y, res


def kernel(x, token_positions, Wq, Wk, Wv, Wo):
    y, _ = run_sharded(x, token_positions, Wq, Wk, Wv, Wo)
    return y


def bench_exec(x, token_positions, Wq, Wk, Wv, Wo, iters=5):
    """Steady-state per-call latency of the compiled 8-core executable with
    device-resident inputs (upper bound on HW exec time: includes per-call
    dispatch overhead).

    Executions are enqueued asynchronously (the per-core NRT queue
    serializes them on-device) and timed in bulk; the per-call time is the
    slope between two batch sizes, which cancels the fixed axon round-trip
    that would otherwise dominate a blocking per-call measurement."""
    import time

    import jax
    import concourse.mybir as mybir
    from concourse import bass2jax
    from jax.sharding import Mesh, NamedSharding, PartitionSpec
    from jax.experimental.shard_map import shard_map

    nc = _get_program()
    in_maps = _host_inputs(x, token_positions, Wq, Wk, Wv, Wo)

    partition_name = (
        nc.partition_id_tensor.name if nc.partition_id_tensor else None
    )
    in_names, out_names, out_avals, zero_outs = [], [], [], []
    for alloc in nc.m.functions[0].allocations:
        if not isinstance(alloc, mybir.MemoryLocationSet):
            continue
        name = alloc.memorylocations[0].name
        if alloc.kind == "ExternalInput":
            if name != partition_name:
                in_names.append(name)
        elif alloc.kind == "ExternalOutput":
            shape = tuple(alloc.tensor_shape)
            dtype = mybir.dt.np(alloc.dtype)
            out_names.append(name)
            out_avals.append(jax.core.ShapedArray(shape, dtype))
            zero_outs.append(np.zeros(shape, dtype))
    n_params = len(in_names)
    all_in = in_names + out_names + ([partition_name] if partition_name else [])

    def _body(*args):
        operands = list(args)
        if partition_name is not None:
            operands.append(bass2jax.partition_id_tensor())
        return tuple(
            bass2jax._bass_exec_p.bind(
                *operands,
                out_avals=tuple(out_avals),
                in_names=tuple(all_in),
                out_names=tuple(out_names),
                lowering_input_output_aliases=(),
                sim_require_finite=True,
                sim_require_nnan=True,
                nc=nc,
            )
        )

    devices = jax.devices()[:NCORES]
    mesh = Mesh(np.asarray(devices), ("core",))
    spec = PartitionSpec("core")
    n_in = n_params + len(out_names)
    fn = jax.jit(
        shard_map(
            _body,
            mesh=mesh,
            in_specs=(spec,) * n_in,
            out_specs=(spec,) * len(out_names),
            check_rep=False,
        ),
        keep_unused=True,
    )
    sharding = NamedSharding(mesh, spec)
    args = [
        jax.device_put(
            np.concatenate([np.asarray(in_maps[c][n]) for c in range(NCORES)], 0),
            sharding,
        )
        for n in in_names
    ] + [
        jax.device_put(
            np.zeros((NCORES * z.shape[0], *z.shape[1:]), z.dtype), sharding
        )
        for z in zero_outs
    ]
    out = fn(*args)
    jax.block_until_ready(out)

    def timed(n):
        t0 = time.time()
        outs = [fn(*args) for _ in range(n)]
        jax.block_until_ready(outs)
        return time.time() - t0

    n1, n2 = 15, 115
    slopes = []
    for _ in range(max(iters, 7)):
        t1 = timed(n1)
        t2 = timed(n2)
        slopes.append((t2 - t1) / (n2 - n1))
    slopes.sort()
    per_call = slopes[len(slopes) // 2]
    if per_call <= 0:  # network jitter swamped the slope; fall back to bulk
        per_call = timed(n2) / n2
    return per_call, out


# revision 30
# speedup vs baseline: 1.2937x; 1.2937x over previous
"""Multi-head self-attention with RoPE (B=2, S=2048, D=1024, H=16, d_k=64,
causal) on 8 trn2 NeuronCores.

Sharding: core c -> batch c//4, heads [4*(c%4), 4*(c%4)+4). Each core gets
x[b]^T, its 4 heads' slices of Wq/Wk/Wv (output dim) and Wo (input dim),
computes a partial y^T = Wo_slice^T . attn_out^T, and the host sums the 4
partials per batch.

Device kernel (per core; matmul operands bf16 by default, f32 PSUM accum):
  1. QKV projection from x^T (model dim on partitions) producing Q^T/K^T
     (head-d on partitions, 2 heads stacked per 128) and V (seq on
     partitions). RoPE applied to Q^T/K^T as q*cos + R^T(q*sin) where R is a
     signed permutation matmul; the head-d axis is pre-permuted (host side)
     to block-of-32 layout so cos/sin rows are partition-aligned.
  2. Transposed-flash attention per (head, 1024-wide q window), k-outer,
     software-pipelined (scores/exp of tile kt+1 overlap attnV of kt):
     scores^T[k,q] = K_tile^T.T @ Q^T (k on partitions), one exp on ACT
     (scale=1/8) over the valid q range, triangular mask multiply on
     diagonal tiles, then attnV out^T[d,q] += V'[k,:].T @ P^T accumulated in
     two half-window [128,512] PSUM tiles. V' carries a ones column (and,
     for odd heads, 64 leading pad columns) so the softmax denominator
     accumulates in a spare PSUM row and odd heads land on partitions
     64..127 directly. Each half normalizes as soon as its k range
     completes: DVE reciprocal of the denominator row -> SBUF, DMA
     partition-broadcast, one DVE multiply into out^T.
  3. y^T[o,s] = Wo^T.T @ out^T, DMA out.
  Phase-1 work for s-chunks 2,3 is interleaved into attention window 0 and
  phase 3 for window w-1 into window w, so PE fills ACT-bound stretches.
"""
import os
import sys

import numpy as np

sys.path.insert(0, "/opt/trn_rl_repo")

D_MODEL = 1024
NUM_HEADS = 16
DK = 64
B = 2
S = 2048
THETA = 10000.0
NCORES = 8
HPC = 4          # heads per core
NPAIRS = 2       # head pairs per core
KT = 128         # k tile (partition dim of scores^T)
QW = 1024        # q window
NW = S // QW     # q windows
NI = D_MODEL // 128   # i (contraction) tiles for projections
NCHUNK = S // 512     # 512-wide s chunks

# V tile layout per head group: [65 | 128 | 65 | 128] columns.
# Even local heads: 64 d columns then a ones column (denominator lands in
# PSUM row 64). Odd local heads: 32 zero cols, ones col, 31 zero cols, then
# 64 d columns -- so attnV output rows are 64..127 (matching oT's lower
# half) and the denominator lands in (32-aligned) PSUM row 32.
VW = 386
V_SLICE = ((0, 65), (65, 193), (193, 258), (258, 386))
V_DEN_ROW = (64, 32)  # PSUM row holding the denominator, per half

_prog = {}


def _mm_mode():
    return os.environ.get("MHA_MM_DTYPE", "bf16")


def _install_hook_wrapper(bass2jax):
    """Install the neuronx compile hook with a traceback printer (the PJRT
    layer swallows python exceptions from the hook)."""
    import traceback

    bass2jax.install_neuronx_cc_hook()
    import libneuronxla

    if getattr(libneuronxla, "_mha_wrapped", False):
        return
    orig = libneuronxla.neuronx_cc

    def wrapped(*a, **k):
        try:
            return orig(*a, **k)
        except Exception:
            traceback.print_exc()
            raise

    libneuronxla.neuronx_cc = wrapped
    libneuronxla._mha_wrapped = True
    bass2jax.install_neuronx_cc_hook = lambda: None


def _split_excess_waits(nc, max_waits=1):
    """This container's walrus accepts at most one sync-wait per
    instruction; redistribute extras onto same-engine NOPs inserted just
    before the offending instruction."""
    import bass_rust
    import concourse.mybir as mybir

    counter = [0]
    for fn in nc.m.functions:
        for bb in fn.blocks:
            out = []
            changed = False
            for inst in bb.instructions:
                si = inst.sync_info
                waits = list(si.on_wait) if si is not None and si.on_wait else []
                if len(waits) > max_waits:
                    changed = True
                    keep = waits[-max_waits:]
                    extras = waits[:-max_waits]
                    for i in range(0, len(extras), max_waits):
                        counter[0] += 1
                        nop = mybir.InstNoOp(
                            name=f"I-waitsplit-{counter[0]}",
                            ins=[],
                            outs=[],
                            engine=inst.engine,
                        )
                        nop.sync_info = bass_rust.SyncInfo(
                            on_wait=extras[i : i + max_waits], on_update=[]
                        )
                        out.append(nop)
                    si.on_wait = keep
                out.append(inst)
            if changed:
                bb.instructions = out


def _build_program(split_waits=True):
    import concourse.bass as bass
    import concourse.mybir as mybir
    from concourse import tile

    F32 = mybir.dt.float32
    mode = _mm_mode()
    MM = {
        "bf16": mybir.dt.bfloat16,
        "f32r": mybir.dt.float32r,
        "f32": mybir.dt.float32,
    }[mode]
    AF = mybir.ActivationFunctionType
    ALU = mybir.AluOpType

    nc = bass.Bass(target_bir_lowering=False, trn_type="TRN2")

    F16 = mybir.dt.float16
    xt = nc.dram_tensor("xt", [D_MODEL, S], MM, kind="ExternalInput")
    # wq/wk/wv packed i-major: [:, 256*i : 256*(i+1)] is contraction tile i,
    # making each weight load a single fully-contiguous DMA
    wqt = nc.dram_tensor("wqt", [128, 2048], MM, kind="ExternalInput")
    wkt = nc.dram_tensor("wkt", [128, 2048], MM, kind="ExternalInput")
    wvt = nc.dram_tensor("wvt", [128, 2048], MM, kind="ExternalInput")
    # cos | sin | rsign | masku packed into one [128, 4352] tensor; Wo^T
    # packed to [128, 2048] (pair-major) -- single contiguous DMAs
    wot = nc.dram_tensor("wot", [128, 2 * D_MODEL], MM, kind="ExternalInput")
    csb = nc.dram_tensor("csb", [128, 2 * S + 256], MM, kind="ExternalInput")
    yt = nc.dram_tensor("yt", [D_MODEL, S], F16, kind="ExternalOutput")

    with tile.TileContext(nc) as tc:
        with (
            tc.tile_pool(name="const", bufs=1) as cp,
            tc.tile_pool(name="xtp", bufs=24) as xtp,
            tc.tile_pool(name="work", bufs=3) as wk,
            tc.tile_pool(name="nrm", bufs=4) as nrm,
            tc.tile_pool(name="bcp", bufs=4) as bcp,
            tc.tile_pool(name="pT", bufs=4) as pTp,
            tc.tile_pool(name="yp", bufs=5) as yp,
            tc.tile_pool(name="psS", bufs=2, space="PSUM") as psS,
            tc.tile_pool(name="psW", bufs=4, space="PSUM") as psW,
        ):
            # ---- DMAs: wq & x chunk 0 first (first-matmul critical path),
            # then rope consts, wk, wv, mask, wo; x chunks 1-3 prefetch ----
            w_pack = {}
            def in_dma(n, out, in_):
                nc.sync.dma_start(out=out, in_=in_)

            wq_sb = cp.tile([128, 2048], MM, tag="wq")
            in_dma(0, wq_sb[:, 0:1024], wqt[:, 0:1024])
            in_dma(0, wq_sb[:, 1024:2048], wqt[:, 1024:2048])
            w_pack["q"] = wq_sb
            wk_early = True
            # x loaded as [128,1024] halves (2KB DMA lines); chunk views below
            x_half = [[None] * NI for _ in range(2)]
            cs_t = cp.tile([128, 2 * S + 256], MM, tag="csb")
            cos_sb = cs_t[:, 0:S]
            sin_sb = cs_t[:, S : 2 * S]
            r_sb = cs_t[:, 2 * S : 2 * S + 128]
            m_sb = cs_t[:, 2 * S + 128 : 2 * S + 256]
            wk_sb = cp.tile([128, 2048], MM, tag="wk")
            n_dma = 1
            in_dma(n_dma, wk_sb[:], wkt[:])
            n_dma += 1
            for i in range(NI):
                t = xtp.tile([128, 1024], MM, tag="xt", name="xh0")
                in_dma(n_dma, t[:], xt[128 * i : 128 * i + 128, 0:1024])
                x_half[0][i] = t
                n_dma += 1
            in_dma(n_dma, cs_t[:], csb[:])
            n_dma += 1
            w_pack["k"] = wk_sb
            wv_sb = cp.tile([128, 2048], MM, tag="wv")
            in_dma(n_dma, wv_sb[:], wvt[:])
            n_dma += 1
            w_pack["v"] = wv_sb
            wo_t = cp.tile([128, 2 * D_MODEL], MM, tag="wo")
            in_dma(n_dma, wo_t[:], wot[:])
            n_dma += 1
            wo_sb = [wo_t[:, D_MODEL * p : D_MODEL * p + D_MODEL] for p in range(NPAIRS)]
            for i in range(NI):
                t = xtp.tile([128, 1024], MM, tag="xt", name="xh1")
                in_dma(n_dma, t[:], xt[128 * i : 128 * i + 128, 1024:2048])
                n_dma += 1
                x_half[1][i] = t

            def w_tile(name, i):
                return w_pack[name][:, 256 * i : 256 * i + 256]

            def x_tile(c, i):
                return x_half[c // 2][i][:, 512 * (c % 2) : 512 * (c % 2) + 512]

            qT_sb = [cp.tile([128, S], MM, tag=f"qT{p}", name=f"qT{p}") for p in range(NPAIRS)]
            kT_sb = [cp.tile([128, S], MM, tag=f"kT{p}", name=f"kT{p}") for p in range(NPAIRS)]
            oT_sb = [cp.tile([128, S], MM, tag=f"oT{p}", name=f"oT{p}") for p in range(NPAIRS)]
            v_sb = [cp.tile([128, VW], MM, tag=f"v{j}", name=f"v{j}") for j in range(S // KT)]

            # ---- phase 1 pieces ----
            def qk_chunk(c, p, name, dst):
                sc = slice(512 * c, 512 * c + 512)
                pc = slice(128 * p, 128 * p + 128)
                ps = psW.tile([128, 512], F32, tag="w")
                for i in range(NI):
                    nc.tensor.matmul(
                        out=ps[:],
                        lhsT=w_tile(name, i)[:, pc],
                        rhs=x_tile(c, i),
                        start=(i == 0),
                        stop=(i == NI - 1),
                    )
                tsin = wk.tile([128, 512], MM, tag="tsin")
                nc.vector.tensor_tensor(
                    out=tsin[:], in0=ps[:], in1=sin_sb[:, sc], op=ALU.mult
                )
                tcos = wk.tile([128, 512], F32, tag="tcos")
                nc.vector.tensor_tensor(
                    out=tcos[:], in0=ps[:], in1=cos_sb[:, sc], op=ALU.mult
                )
                pssh = psW.tile([128, 512], F32, tag="w")
                nc.tensor.matmul(
                    out=pssh[:], lhsT=r_sb, rhs=tsin[:], start=True, stop=True
                )
                nc.vector.tensor_tensor(
                    out=dst[p][:, sc], in0=pssh[:], in1=tcos[:], op=ALU.add
                )

            def v_chunk_tile(c, st):
                j = 4 * c + st
                stl = slice(128 * st, 128 * st + 128)
                psv = psW.tile([128, 256], F32, tag="w")
                for i in range(NI):
                    nc.tensor.matmul(
                        out=psv[:],
                        lhsT=x_tile(c, i)[:, stl],
                        rhs=w_tile("v", i),
                        start=(i == 0),
                        stop=(i == NI - 1),
                    )
                vt = v_sb[j]
                base = vt[:]
                # odd-head prefix: zeros (64 cols), then ones at local col 32
                nc.vector.memset(
                    bass.AP(base.tensor, base.offset + 65, [[VW, 128], [193, 2], [1, 64]]),
                    0.0,
                )
                nc.vector.memset(
                    bass.AP(base.tensor, base.offset + 97, [[VW, 128], [193, 2]]), 1.0
                )
                # even-head ones column (col 64 of the 65-wide slices)
                nc.vector.memset(
                    bass.AP(base.tensor, base.offset + 64, [[VW, 128], [193, 2]]), 1.0
                )
                pv = psv[:]
                # d columns: even halves (offsets 0, 193), odd halves (129, 322)
                nc.vector.tensor_copy(
                    out=bass.AP(base.tensor, base.offset + 0, [[VW, 128], [193, 2], [1, 64]]),
                    in_=bass.AP(pv.tensor, pv.offset + 0, [[256, 128], [128, 2], [1, 64]]),
                )
                nc.vector.tensor_copy(
                    out=bass.AP(base.tensor, base.offset + 129, [[VW, 128], [193, 2], [1, 64]]),
                    in_=bass.AP(pv.tensor, pv.offset + 64, [[256, 128], [128, 2], [1, 64]]),
                )

            # ---- attention ----
            def normalize(w, h, acc_t, beta):
                p, half = divmod(h, 2)
                qs = slice(QW * w + 512 * beta, QW * w + 512 * beta + 512)
                dr = 64 * half  # d-row base in acc/oT
                den = nrm.tile([1, 512], F32, tag="den")
                drow = V_DEN_ROW[half]
                nc.vector.reciprocal(
                    out=den[:], in_=acc_t[drow : drow + 1, :]
                )
                bc = bcp.tile([128, 512], F32, tag="bc")
                dap = den[:]
                nc.gpsimd.dma_start(
                    out=bc[dr : dr + 64, :],
                    in_=bass.AP(dap.tensor, dap.offset, [[512, 1], [0, 64], [1, 512]]),
                )
                nc.vector.tensor_tensor(
                    out=oT_sb[p][dr : dr + 64, qs],
                    in0=acc_t[dr : dr + 64, :],
                    in1=bc[dr : dr + 64, :],
                    op=ALU.mult,
                )

            def attn_head(w, h, post_left=None, post_right=None):
                p, half = divmod(h, 2)
                pr = slice(64 * half, 64 * half + 64)
                a0, a1 = V_SLICE[h]
                q0 = QW * w
                acc = [
                    psW.tile([128, 512], F32, tag="w", name="accL"),
                    psW.tile([128, 512], F32, tag="w", name="accR"),
                ]
                kmax = (QW // KT) * (w + 1)
                left_stop = (QW // KT) * w + 3
                pend = None  # software pipeline: attnV trails scores/exp by one

                def attn_v(kt, pT):
                    k0 = KT * kt
                    qoff = max(k0 - q0, 0)
                    subs = [(qoff, 512), (512, QW)] if qoff < 512 else [(qoff, QW)]
                    for a, b in subs:
                        beta = a // 512
                        nc.tensor.matmul(
                            out=acc[beta][0 : a1 - a0, a - 512 * beta : b - 512 * beta],
                            lhsT=v_sb[kt][:, a0:a1],
                            rhs=pT[:, a:b],
                            start=(kt == 0),
                            stop=(kt == left_stop + 4 * beta),
                        )
                    if kt == left_stop:
                        normalize(w, h, acc[0], 0)
                        if post_left:
                            post_left()

                for kt in range(kmax):
                    k0 = KT * kt
                    qoff = max(k0 - q0, 0)
                    ps_s = psS.tile([128, QW], F32, tag="s")
                    subs = [(qoff, 512), (512, QW)] if qoff < 512 else [(qoff, QW)]
                    for a, b in subs:
                        nc.tensor.matmul(
                            out=ps_s[:, a:b],
                            lhsT=kT_sb[p][pr, k0 : k0 + KT],
                            rhs=qT_sb[p][pr, q0 + a : q0 + b],
                            start=True,
                            stop=True,
                        )
                    pT = pTp.tile([128, QW], MM, tag="pT")
                    nc.scalar.activation(
                        out=pT[:, qoff:QW], in_=ps_s[:, qoff:QW], func=AF.Exp, scale=0.125
                    )
                    if k0 >= q0:
                        nc.vector.tensor_tensor(
                            out=pT[:, qoff : qoff + KT],
                            in0=pT[:, qoff : qoff + KT],
                            in1=m_sb,
                            op=ALU.mult,
                        )
                    if pend is not None:
                        attn_v(*pend)
                    pend = (kt, pT)
                attn_v(*pend)
                normalize(w, h, acc[1], 1)
                if post_right:
                    post_right()

            # ---- phase 3 ----
            def emit_phase3_chunk(c, ocr, use_psS, act_alt):
                # phase 3 for 512-col chunk c, output rows in `ocr`.
                # use_psS: also draw PSUM slots from the (drained) scores
                # pool. act_alt: alternate copies ACT/DVE (else DVE only --
                # used while ACT is still exp-bound).
                sc = slice(512 * c, 512 * c + 512)
                for n_item, oc in enumerate(ocr):
                    ocs = slice(128 * oc, 128 * oc + 128)
                    if use_psS and n_item % 3 == 2:
                        ps_y = psS.tile([128, 512], F32, tag="s", name="ps_ys")
                    else:
                        ps_y = psW.tile([128, 512], F32, tag="w", name="ps_y")
                    for p in range(NPAIRS):
                        nc.tensor.matmul(
                            out=ps_y[:],
                            lhsT=wo_sb[p][:, ocs],
                            rhs=oT_sb[p][:, sc],
                            start=(p == 0),
                            stop=(p == NPAIRS - 1),
                        )
                    y_sb = yp.tile([128, 512], F16, tag="y", name="y_sb")
                    if act_alt and n_item % 2 == 0:
                        nc.scalar.activation(out=y_sb[:], in_=ps_y[:], func=AF.Copy)
                    else:
                        nc.vector.tensor_copy(out=y_sb[:], in_=ps_y[:])
                    nc.sync.dma_start(out=yt[ocs, sc], in_=y_sb[:])

            def emit_phase3(win, part=None, chunks=None, items=None, act_copy=False):
                if items is None:
                    items = [
                        (c, oc)
                        for c in (chunks if chunks is not None else (2 * win, 2 * win + 1))
                        for oc in range(D_MODEL // 128)
                    ]
                    if part is not None:
                        items = items[4 * part : 4 * part + 4]
                for n_item, (c, oc) in enumerate(items):
                    sc = slice(512 * c, 512 * c + 512)
                    ocs = slice(128 * oc, 128 * oc + 128)
                    ps_y = psW.tile([128, 512], F32, tag="w", name="ps_y")
                    for p in range(NPAIRS):
                        nc.tensor.matmul(
                            out=ps_y[:],
                            lhsT=wo_sb[p][:, ocs],
                            rhs=oT_sb[p][:, sc],
                            start=(p == 0),
                            stop=(p == NPAIRS - 1),
                        )
                    y_sb = yp.tile([128, 512], F16, tag="y", name="y_sb")
                    if act_copy and n_item % 2 == 0:
                        nc.scalar.activation(
                            out=y_sb[:], in_=ps_y[:], func=AF.Copy
                        )
                    else:
                        nc.vector.tensor_copy(out=y_sb[:], in_=ps_y[:])
                    nc.sync.dma_start(out=yt[ocs, sc], in_=y_sb[:])

            # ---- schedule ----
            for c in (0, 1):
                for p in range(NPAIRS):
                    qk_chunk(c, p, "q", qT_sb)
                    qk_chunk(c, p, "k", kT_sb)
                for st in range(4):
                    v_chunk_tile(c, st)

            def fill(idx):
                # phase-1 work for chunks 2,3, slotted into window-0 attention
                if idx == 0:
                    qk_chunk(2, 0, "q", qT_sb)
                    qk_chunk(2, 0, "k", kT_sb)
                elif idx == 1:
                    qk_chunk(2, 1, "q", qT_sb)
                    qk_chunk(2, 1, "k", kT_sb)
                    for st in range(4):
                        v_chunk_tile(2, st)
                elif idx == 2:
                    qk_chunk(3, 0, "q", qT_sb)
                    qk_chunk(3, 0, "k", kT_sb)
                    for st in range(4):
                        v_chunk_tile(3, st)
                else:
                    qk_chunk(3, 1, "q", qT_sb)
                    qk_chunk(3, 1, "k", kT_sb)

            horder = (1, 3, 0, 2)
            for idx, h in enumerate(horder):
                attn_head(0, h)
                fill(idx)
            p30 = [(c, oc) for c in (0, 1) for oc in range(D_MODEL // 128)]
            for idx, h in enumerate(horder):
                last = idx == len(horder) - 1
                mine = p30[4 * idx : 4 * idx + 4]

                def post_left(mine=mine):
                    emit_phase3(0, items=mine[:2])

                def post_right(mine=mine, last=last):
                    emit_phase3(0, items=mine[2:])
                    if last:
                        emit_phase3_chunk(2, range(8), use_psS=True, act_alt=True)

                attn_head(1, h, post_left=post_left, post_right=post_right)
            emit_phase3_chunk(3, range(8), use_psS=True, act_alt=True)

    if split_waits:
        _split_excess_waits(nc)
    return nc


def _get_program():
    if "nc" not in _prog:
        from concourse import bass2jax

        _install_hook_wrapper(bass2jax)
        _prog["nc"] = _build_program()
    return _prog["nc"]


def _perm_rows(g):
    """DRAM row order of Wq/Wk for core head-group g: pair-major, head-major,
    evens-then-odds within each head's 64 dims."""
    perm64 = list(range(0, 64, 2)) + list(range(1, 64, 2))
    rows = []
    for h in range(HPC):
        head = HPC * g + h
        rows += [64 * head + j for j in perm64]
    return rows


def _plain_rows(g):
    return [64 * (HPC * g) + j for j in range(64 * HPC)]


def _np_mm():
    if _mm_mode() == "bf16":
        import ml_dtypes

        return ml_dtypes.bfloat16
    return np.float32


def _host_inputs(x, token_positions, Wq, Wk, Wv, Wo):
    mmt = _np_mm()
    x = np.asarray(x, dtype=np.float32)
    pos = np.asarray(token_positions).astype(np.float64)
    Wq = np.asarray(Wq, dtype=np.float32)
    Wk = np.asarray(Wk, dtype=np.float32)
    Wv = np.asarray(Wv, dtype=np.float32)
    Wo = np.asarray(Wo, dtype=np.float32)

    inv = 1.0 / THETA ** (np.arange(0, DK, 2, dtype=np.float64) / DK)
    ang = pos[:, None] * inv[None, :]          # (S, 32)
    cosb = np.tile(np.cos(ang).T.astype(np.float32), (4, 1))  # (128, S)
    sinb = np.tile(np.sin(ang).T.astype(np.float32), (4, 1))

    rsign = np.zeros((128, 128), dtype=np.float32)
    j = np.arange(32)
    for blk in range(2):
        o = 64 * blk
        rsign[o + 32 + j, o + j] = -1.0
        rsign[o + j, o + 32 + j] = 1.0
    masku = np.triu(np.ones((128, 128), dtype=np.float32))

    def _pack(wt):  # (1024, 256) -> (128, 2048), i-major contraction tiles
        return np.ascontiguousarray(
            wt.reshape(8, 128, 256).transpose(1, 0, 2).reshape(128, 2048)
        )

    in_maps = []
    for c in range(NCORES):
        b, g = divmod(c, 4)
        rows = _perm_rows(g)
        vrows = _plain_rows(g)
        in_maps.append(
            {
                "xt": np.ascontiguousarray(x[b].T).astype(mmt),
                "wqt": _pack(Wq[rows, :].T).astype(mmt),
                "wkt": _pack(Wk[rows, :].T).astype(mmt),
                "wvt": _pack(Wv[vrows, :].T).astype(mmt),
                "wot": np.ascontiguousarray(
                    np.concatenate(
                        [Wo[:, vrows].T[128 * p : 128 * p + 128, :] for p in range(2)],
                        axis=1,
                    )
                ).astype(mmt),
                "csb": np.ascontiguousarray(
                    np.concatenate([cosb, sinb, rsign, masku], axis=1)
                ).astype(mmt),
            }
        )
    return in_maps


def run_sharded(x, token_positions, Wq, Wk, Wv, Wo, trace=False):
    from concourse.bass_utils import run_bass_kernel_spmd

    nc = _get_program()
    in_maps = _host_inputs(x, token_positions, Wq, Wk, Wv, Wo)
    res = run_bass_kernel_spmd(
        nc, in_maps, list(range(NCORES)), trace=trace
    )
    y = np.zeros((B, S, D_MODEL), dtype=np.float32)
    for c in range(NCORES):
        y[c // 4] += res.results[c]["yt"].T.astype(np.float32)
    return y, res


def kernel(x, token_positions, Wq, Wk, Wv, Wo):
    y, _ = run_sharded(x, token_positions, Wq, Wk, Wv, Wo)
    return y


def bench_exec(x, token_positions, Wq, Wk, Wv, Wo, iters=5):
    """Steady-state per-call latency of the compiled 8-core executable with
    device-resident inputs (upper bound on HW exec time: includes per-call
    dispatch overhead).

    Executions are enqueued asynchronously (the per-core NRT queue
    serializes them on-device) and timed in bulk; the per-call time is the
    slope between two batch sizes, which cancels the fixed axon round-trip
    that would otherwise dominate a blocking per-call measurement."""
    import time

    import jax
    import concourse.mybir as mybir
    from concourse import bass2jax
    from jax.sharding import Mesh, NamedSharding, PartitionSpec
    from jax.experimental.shard_map import shard_map

    nc = _get_program()
    in_maps = _host_inputs(x, token_positions, Wq, Wk, Wv, Wo)

    partition_name = (
        nc.partition_id_tensor.name if nc.partition_id_tensor else None
    )
    in_names, out_names, out_avals, zero_outs = [], [], [], []
    for alloc in nc.m.functions[0].allocations:
        if not isinstance(alloc, mybir.MemoryLocationSet):
            continue
        name = alloc.memorylocations[0].name
        if alloc.kind == "ExternalInput":
            if name != partition_name:
                in_names.append(name)
        elif alloc.kind == "ExternalOutput":
            shape = tuple(alloc.tensor_shape)
            dtype = mybir.dt.np(alloc.dtype)
            out_names.append(name)
            out_avals.append(jax.core.ShapedArray(shape, dtype))
            zero_outs.append(np.zeros(shape, dtype))
    n_params = len(in_names)
    all_in = in_names + out_names + ([partition_name] if partition_name else [])

    def _body(*args):
        operands = list(args)
        if partition_name is not None:
            operands.append(bass2jax.partition_id_tensor())
        return tuple(
            bass2jax._bass_exec_p.bind(
                *operands,
                out_avals=tuple(out_avals),
                in_names=tuple(all_in),
                out_names=tuple(out_names),
                lowering_input_output_aliases=(),
                sim_require_finite=True,
                sim_require_nnan=True,
                nc=nc,
            )
        )

    devices = jax.devices()[:NCORES]
    mesh = Mesh(np.asarray(devices), ("core",))
    spec = PartitionSpec("core")
    n_in = n_params + len(out_names)
    fn = jax.jit(
        shard_map(
            _body,
            mesh=mesh,
            in_specs=(spec,) * n_in,
            out_specs=(spec,) * len(out_names),
            check_rep=False,
        ),
        keep_unused=True,
    )
    sharding = NamedSharding(mesh, spec)
    args = [
        jax.device_put(
            np.concatenate([np.asarray(in_maps[c][n]) for c in range(NCORES)], 0),
            sharding,
        )
        for n in in_names
    ] + [
        jax.device_put(
            np.zeros((NCORES * z.shape[0], *z.shape[1:]), z.dtype), sharding
        )
        for z in zero_outs
    ]
    out = fn(*args)
    jax.block_until_ready(out)

    def timed(n):
        t0 = time.time()
        outs = [fn(*args) for _ in range(n)]
        jax.block_until_ready(outs)
        return time.time() - t0

    n1, n2 = 15, 140
    slopes = []
    for _ in range(max(iters, 11)):
        t1 = timed(n1)
        t2 = timed(n2)
        slopes.append((t2 - t1) / (n2 - n1))
    slopes.sort()
    per_call = slopes[len(slopes) // 2]
    if per_call <= 0:  # network jitter swamped the slope; fall back to bulk
        per_call = timed(n2) / n2
    return per_call, out
